# revision 9
# baseline (speedup 1.0000x reference)
"""Trainium2 Bass kernel for nn_MultiHeadAttention_3539053052118.

GQA attention (B=2, S=2048, HID=2048, 16 q-heads, 4 kv-heads, RoPE, causal)
distributed over 8 NeuronCores: 2-way data-parallel over batch x 4-way
tensor-parallel over kv-head groups. Each core computes q/kv projections for
its 4 q-heads + 1 kv-head (fp32r matmuls), RoPE, causal flash attention; each
head's context is AllGather-ed (bf16) within the 4-core batch group as soon
as it is ready, and the o_proj accumulates per-wave into SBUF so the
collectives overlap attention. Each core produces a distinct 512-column slice
of the output. The host only shards/aliases inputs and concatenates slices.
"""

import math
import sys
import types

sys.path.insert(0, "/opt/trn_rl_repo")

import antenv  # noqa: F401

if "antenv.axon_hooks" not in sys.modules:
    _hooks = types.ModuleType("antenv.axon_hooks")
    _hook_box = {"hook": None}
    _hooks.set_axon_ntff_profile_hook = lambda h: _hook_box.__setitem__("hook", h)
    _hooks.get_axon_ntff_profile_hook = lambda: _hook_box["hook"]
    sys.modules["antenv.axon_hooks"] = _hooks
    try:
        from trn_agent_boot.trn_boot import _ntff_profile_via_ctypes

        _hooks.set_axon_ntff_profile_hook(
            _ntff_profile_via_ctypes("/opt/axon/libaxon_pjrt.so")
        )
    except Exception:
        pass

import numpy as np
import concourse.bass as bass
import concourse.mybir as mybir
import concourse.tile as tile
from concourse import bacc
from concourse import bass_utils
from concourse.masks import make_identity

F32 = mybir.dt.float32
F32R = mybir.dt.float32r
BF16 = mybir.dt.bfloat16
I32 = mybir.dt.int32
AF = mybir.ActivationFunctionType
ALU = mybir.AluOpType

B, S, HID = 2, 2048, 2048
NH, NKV = 16, 4
HD = 128
ROPE_BASE = 10000.0
PI = math.pi

N_CORES = 8
TP = 4
HG = NH // TP  # 4 q heads per core
GROUPS = [[0, 1, 2, 3], [4, 5, 6, 7]]

NKC = HID // 128  # 16 contraction tiles
NQC = S // 512  # 4 q/n chunks
NST = S // 128  # 16 s tiles
OC = 512  # output columns per core

_CACHE = {}


def _build():
    nc = bacc.Bacc("TRN2", target_bir_lowering=False, debug=False, num_devices=N_CORES)

    xT = nc.dram_tensor("xT", [HID, S], F32R, kind="ExternalInput").ap()
    wqT = nc.dram_tensor("wqT", [HID, HG * HD], F32R, kind="ExternalInput").ap()
    wkT = nc.dram_tensor("wkT", [HID, HD], F32R, kind="ExternalInput").ap()
    wvT = nc.dram_tensor("wvT", [HID, HD], F32R, kind="ExternalInput").ap()
    woT = nc.dram_tensor("woT", [HID, OC], F32, kind="ExternalInput").ap()
    pos = nc.dram_tensor("pos", [1, S], I32, kind="ExternalInput").ap()
    out = nc.dram_tensor("out_slice", [S, OC], F32, kind="ExternalOutput").ap()

    # per-head collective bounce buffers (separate tensors so AG(h) only
    # depends on head h's writes)
    cc_in = [
        [nc.dram_tensor(f"cc_in{h}_{hf}", [HD, S // 2], BF16).ap() for hf in range(2)]
        for h in range(HG)
    ]
    cc_out = [
        [
            nc.dram_tensor(f"cc_out{h}_{hf}", [TP * HD, S // 2], BF16).ap()
            for hf in range(2)
        ]
        for h in range(HG)
    ]

    # ---- inline constants ----
    half = HD // 2
    invf = 1.0 / (ROPE_BASE ** (np.arange(half) / half))
    invf_t = np.tile(invf, 2)[:, None].astype(np.float32) / (2 * PI)  # turns
    invf_c = nc.inline_tensor(invf_t, "invf").ap()
    R = np.zeros((HD, HD), np.float32)
    for p in range(half):
        R[p, p + half] = -1.0
    for p in range(half, HD):
        R[p, p - half] = 1.0
    permRT_c = nc.inline_tensor(np.ascontiguousarray(R.T), "permRT").ap()
    ones_row_c = nc.inline_tensor(np.ones((1, 128), np.float32), "ones_row").ap()
    import ml_dtypes

    ones_col_c = nc.inline_tensor(
        np.ones((128, 1), ml_dtypes.bfloat16), "ones_col"
    ).ap()
    # causal bias: B[p, j] = -30000 where key p > query j (within diag subtile)
    btri = np.where(
        np.arange(128)[:, None] > np.arange(128)[None, :], -30000.0, 0.0
    ).astype(np.float32)
    btriT_c = nc.inline_tensor(np.ascontiguousarray(btri.T), "btriT").ap()
    iext_np = np.zeros((128, 512), np.float32)
    iext_np[:, :128] = np.eye(128, dtype=np.float32)
    iext_c = nc.inline_tensor(iext_np, "iext").ap()

    with tile.TileContext(nc) as tc:
        with (
            tc.tile_pool(name="const", bufs=1) as cpool,
            tc.tile_pool(name="w", bufs=1) as wpool,
            tc.tile_pool(name="attn", bufs=2) as apool,
        ):
            qkvpool = tc.alloc_tile_pool(name="qkv", bufs=1)
            # ---- constants ----
            invf_sb = cpool.tile([HD, 1], F32)
            nc.sync.dma_start(out=invf_sb[:, :], in_=invf_c[:, :])
            permRT_sb = cpool.tile([HD, HD], F32R)
            nc.sync.dma_start(out=permRT_sb[:, :], in_=permRT_c.bitcast(F32R)[:, :])
            ones_row_sb = cpool.tile([1, 128], F32R)
            nc.sync.dma_start(out=ones_row_sb[:, :], in_=ones_row_c.bitcast(F32R)[:, :])
            ones_col_sb = cpool.tile([128, 1], BF16)
            nc.sync.dma_start(out=ones_col_sb[:, :], in_=ones_col_c[:, :])
            btriT_sb = cpool.tile([128, 128], F32R)
            nc.sync.dma_start(out=btriT_sb[:, :], in_=btriT_c.bitcast(F32R)[:, :])
            iext_sb = cpool.tile([128, 512], F32R)
            nc.sync.dma_start(out=iext_sb[:, :], in_=iext_c.bitcast(F32R)[:, :])
            ident_sb = cpool.tile([128, 128], F32)
            make_identity(nc, ident_sb[:, :])

            # ---- phase 0: rope tables (chunked temps) ----
            psT = tc.alloc_tile_pool(name="psT", bufs=2, space="PSUM")
            tabtmp = tc.alloc_tile_pool(name="tabtmp", bufs=2)
            sinT = cpool.tile([128, S], F32, tag="tab_sin", name="tab_sin")
            cosT = cpool.tile([128, S], F32, tag="tab_cos", name="tab_cos")
            for q in range(NQC):
                ns = slice(q * 512, (q + 1) * 512)
                pos_i = tabtmp.tile([1, 512], I32, tag="pos_i", name=f"pos_i{q}")
                nc.sync.dma_start(out=pos_i[:, :], in_=pos[:, ns])
                pos_f = tabtmp.tile([1, 512], F32R, tag="pos_f", name=f"pos_f{q}")
                nc.vector.tensor_copy(pos_f[:, :], pos_i[:, :])
                ps_pos = psT.tile([128, 512], F32, tag="pos", name=f"ps_pos{q}")
                nc.tensor.matmul(
                    ps_pos[:, :], ones_row_sb[:, :], pos_f[:, :], start=True, stop=True
                )
                for add_quarter, tab in ((False, sinT), (True, cosT)):
                    nm = f"{q}_{int(add_quarter)}"
                    t_t = tabtmp.tile([128, 512], F32, tag="t_t", name=f"t_{nm}")
                    if add_quarter:
                        nc.vector.tensor_scalar(
                            t_t[:, :], ps_pos[:, :], invf_sb[:, :], 0.25,
                            op0=ALU.mult, op1=ALU.add,
                        )
                    else:
                        nc.vector.tensor_scalar_mul(
                            t_t[:, :], ps_pos[:, :], invf_sb[:, :]
                        )
                    t_i = tabtmp.tile([128, 512], I32, tag="t_i", name=f"ti_{nm}")
                    nc.vector.tensor_copy(t_i[:, :], t_t[:, :])
                    t_f = tabtmp.tile([128, 512], F32, tag="t_f", name=f"tf_{nm}")
                    nc.vector.tensor_copy(t_f[:, :], t_i[:, :])
                    nc.vector.tensor_sub(t_t[:, :], t_t[:, :], t_f[:, :])
                    nc.scalar.activation(tab[:, ns], t_t[:, :], AF.Sin, scale=2 * PI)
            tabtmp.release()
            psT.release()

            # ---- weights: wk/wv persistent; wo loaded + cast to bf16 ----
            wk_sb = [
                wpool.tile([128, HD], F32R, tag=f"wk{i}", name=f"wk{i}")
                for i in range(NKC)
            ]
            wv_sb = [
                wpool.tile([128, HD], F32R, tag=f"wv{i}", name=f"wv{i}")
                for i in range(NKC)
            ]
            wo_sb = [
                wpool.tile([128, OC], BF16, tag=f"wo{i}", name=f"wo{i}")
                for i in range(NKC)
            ]
            # persistent qkv storage
            q_sb = [
                qkvpool.tile([128, S], F32R, tag=f"q{h}", name=f"q{h}")
                for h in range(HG)
            ]
            k_sb = qkvpool.tile([128, S], F32R, tag="k", name="k_sb")
            vT_sb = qkvpool.tile([128, S], F32, tag="vT", name="vT_sb")
            v_sb = [
                qkvpool.tile([128, HD], BF16, tag=f"v{i}", name=f"v{i}")
                for i in range(NST)
            ]

            wqpool = tc.alloc_tile_pool(name="wq", bufs=1)
            wq_sb = [
                wqpool.tile([128, HG * HD], F32R, tag=f"wq{i}", name=f"wq{i}")
                for i in range(NKC)
            ]
            xspool = tc.alloc_tile_pool(name="xs", bufs=1)
            psP = tc.alloc_tile_pool(name="psP", bufs=1, space="PSUM")
            psR = tc.alloc_tile_pool(name="psR", bufs=1, space="PSUM")

            # prefetch chunk-0 x before the bulk weight loads
            x_pending = {}
            for kt in range(NKC):
                t = xspool.tile([128, 512], F32R, tag=f"x{kt}", name=f"x_0_{kt}")
                nc.sync.dma_start(out=t[:, :], in_=xT[kt * 128 : (kt + 1) * 128, 0:512])
                x_pending[kt] = t
            for kt in range(NKC):
                sl = slice(kt * 128, (kt + 1) * 128)
                nc.sync.dma_start(out=wq_sb[kt][:, :], in_=wqT[sl, :])
                nc.sync.dma_start(out=wk_sb[kt][:, :], in_=wkT[sl, :])
                nc.sync.dma_start(out=wv_sb[kt][:, :], in_=wvT[sl, :])
            for kt in range(NKC):
                wtmp = apool.tile([128, OC], F32, tag="wotmp", name=f"wotmp{kt}")
                nc.sync.dma_start(out=wtmp[:, :], in_=woT[kt * 128 : (kt + 1) * 128, :])
                nc.vector.tensor_copy(wo_sb[kt][:, :], wtmp[:, :])

            # ---- phase 1: projections + rope + v transpose ----
            for q in range(NQC):
                ns = slice(q * 512, (q + 1) * 512)
                if q == 0:
                    x_sb = [x_pending[kt] for kt in range(NKC)]
                else:
                    x_sb = []
                    for kt in range(NKC):
                        t = xspool.tile(
                            [128, 512], F32R, tag=f"x{kt}", name=f"x_{q}_{kt}"
                        )
                        nc.sync.dma_start(
                            out=t[:, :], in_=xT[kt * 128 : (kt + 1) * 128, ns]
                        )
                        x_sb.append(t)
                ps_proj = [
                    psP.tile([128, 512], F32, tag=f"proj{i}", name=f"proj{i}_{q}")
                    for i in range(HG + 2)
                ]
                for kt in range(NKC):
                    st, sp = kt == 0, kt == NKC - 1
                    for h in range(HG):
                        nc.tensor.matmul(
                            ps_proj[h][:, :],
                            wq_sb[kt][:, h * HD : (h + 1) * HD],
                            x_sb[kt][:, :],
                            start=st,
                            stop=sp,
                        )
                    nc.tensor.matmul(
                        ps_proj[HG][:, :], wk_sb[kt][:, :], x_sb[kt][:, :],
                        start=st, stop=sp,
                    )
                    nc.tensor.matmul(
                        ps_proj[HG + 1][:, :], wv_sb[kt][:, :], x_sb[kt][:, :],
                        start=st, stop=sp,
                    )

                for h in range(HG):
                    nc.vector.tensor_copy(q_sb[h][:, ns], ps_proj[h][:, :])
                nc.vector.tensor_copy(k_sb[:, ns], ps_proj[HG][:, :])
                nc.vector.tensor_copy(vT_sb[:, ns], ps_proj[HG + 1][:, :])

                for idx in range(HG + 1):
                    tgt = q_sb[idx][:, ns] if idx < HG else k_sb[:, ns]
                    ps_rot = psR.tile([128, 512], F32, tag="rot", name=f"rot{q}_{idx}")
                    nc.tensor.matmul(
                        ps_rot[:, :], permRT_sb[:, :], tgt, start=True, stop=True
                    )
                    tmp = apool.tile([128, 512], F32, tag="ropetmp", name=f"rt{q}_{idx}")
                    nc.vector.tensor_tensor(tmp[:, :], tgt, cosT[:, ns], op=ALU.mult)
                    nc.vector.tensor_tensor(tgt, ps_rot[:, :], sinT[:, ns], op=ALU.mult)
                    nc.vector.tensor_tensor(tgt, tgt, tmp[:, :], op=ALU.add)

                for j in range(4):
                    stile = q * 4 + j
                    ps_v = psR.tile([128, 128], F32, tag="vt", name=f"vt{stile}")
                    nc.tensor.transpose(
                        ps_v[:, :],
                        vT_sb[:, stile * 128 : (stile + 1) * 128],
                        ident_sb[:, :],
                    )
                    nc.vector.tensor_copy(v_sb[stile][:, :], ps_v[:, :])
            psR.release()
            psP.release()
            xspool.release()
            wqpool.release()

            # ---- phase 2: attention; AG(h) issued per head; o_proj waves ----
            ppool = tc.alloc_tile_pool(name="probs", bufs=3)
            ctxpool = tc.alloc_tile_pool(name="ctx", bufs=2)
            accpool = tc.alloc_tile_pool(name="acc", bufs=1)
            cblkpool = tc.alloc_tile_pool(name="cblk", bufs=1)
            ps2 = tc.alloc_tile_pool(name="ps2", bufs=1, space="PSUM")
            psO = tc.alloc_tile_pool(name="psO", bufs=2, space="PSUM")

            scale = float(HD**-0.5)
            for h in range(HG):
                for q in range(NQC):
                    ns = slice(q * 512, (q + 1) * 512)
                    nkt = 4 * q + 4
                    ps_sums = ps2.tile(
                        [1, 512], F32, tag="sums", name=f"sums{h}_{q}", bufs=2
                    )
                    ps_ctx = ps2.tile(
                        [128, 512], F32, tag="ctx", name=f"ctx{h}_{q}", bufs=2
                    )
                    for kt in range(nkt):
                        o = kt - 4 * q
                        c0 = max(0, o) * 128  # first valid column in the chunk
                        cs = slice(c0, 512)
                        ps_s = ps2.tile(
                            [128, 512], F32, tag="scores", name=f"s{h}_{q}_{kt}", bufs=2
                        )
                        if o >= 0:
                            # causal bias written into the bank, scores accumulate
                            nc.tensor.matmul(
                                ps_s[:, cs],
                                btriT_sb[:, :],
                                iext_sb[:, 0 : 512 - c0],
                                start=True,
                                stop=False,
                            )
                        nc.tensor.matmul(
                            ps_s[:, cs],
                            k_sb[:, kt * 128 : (kt + 1) * 128],
                            q_sb[h][:, q * 512 + c0 : (q + 1) * 512],
                            start=o < 0,
                            stop=True,
                        )
                        pT = ppool.tile(
                            [128, 512], BF16, tag="probs", name=f"p{h}_{q}_{kt}"
                        )
                        nc.scalar.activation(pT[:, cs], ps_s[:, cs], AF.Exp, scale=scale)
                        st, sp = kt == 0, kt == nkt - 1
                        nc.tensor.matmul(
                            ps_sums[:, cs], ones_col_sb[:, :], pT[:, cs],
                            start=st, stop=sp,
                        )
                        nc.tensor.matmul(
                            ps_ctx[:, cs], v_sb[kt][:, :], pT[:, cs],
                            start=st, stop=sp,
                        )
                    # free the psum banks fast; normalize off the PE path
                    ctx_raw = apool.tile(
                        [128, 512], F32, tag="ctx_raw", name=f"cr{h}_{q}"
                    )
                    nc.vector.tensor_copy(ctx_raw[:, :], ps_ctx[:, :])
                    recip = apool.tile([1, 512], F32R, tag="recip", name=f"rc{h}_{q}")
                    with nc.allow_low_precision(reason="softmax recip to f32r"):
                        nc.vector.reciprocal(recip[:, :], ps_sums[:, :])
                    ps_rb = ps2.tile(
                        [128, 512], F32, tag="scores", name=f"rb{h}_{q}", bufs=2
                    )
                    nc.tensor.matmul(
                        ps_rb[:, :], ones_row_sb[:, :], recip[:, :], start=True, stop=True
                    )
                    csb = ctxpool.tile([128, 512], BF16, tag="ctxsb", name=f"cs{h}_{q}")
                    nc.vector.tensor_tensor(
                        csb[:, :], ctx_raw[:, :], ps_rb[:, :], op=ALU.mult
                    )
                    hf = q // 2
                    nc.sync.dma_start(
                        out=cc_in[h][hf][:, (q % 2) * 512 : (q % 2 + 1) * 512],
                        in_=csb[:, :],
                    )
                    if q % 2 == 1:
                        # half of head h's context is done on all ranks
                        nc.gpsimd.collective_compute(
                            "AllGather",
                            mybir.AluOpType.bypass,
                            replica_groups=GROUPS,
                            ins=[cc_in[h][hf][:, :]],
                            outs=[cc_out[h][hf][:, :]],
                        )

            # o_proj waves: wave h brings rows for global kt = 4r + h
            acc_sb = [
                accpool.tile([128, OC], F32, tag=f"acc{i}", name=f"acc{i}")
                for i in range(NST)
            ]
            for h in range(HG):
                for hf in range(2):
                    cblk = []
                    for r in range(TP):
                        t = cblkpool.tile(
                            [128, S // 2], BF16, tag=f"cblk{r}",
                            name=f"cb{h}_{hf}_{r}", bufs=2,
                        )
                        nc.sync.dma_start(
                            out=t[:, :], in_=cc_out[h][hf][r * 128 : (r + 1) * 128, :]
                        )
                        cblk.append(t)
                    for j in range(NST // 2):
                        stile = hf * (NST // 2) + j
                        ps_po = psO.tile(
                            [128, OC], F32, tag="po", name=f"po{h}_{hf}_{j}"
                        )
                        for r in range(TP):
                            nc.tensor.matmul(
                                ps_po[:, :],
                                cblk[r][:, j * 128 : (j + 1) * 128],
                                wo_sb[4 * r + h][:, :],
                                start=r == 0,
                                stop=r == TP - 1,
                            )
                        if h == 0:
                            nc.vector.tensor_copy(acc_sb[stile][:, :], ps_po[:, :])
                        else:
                            nc.vector.tensor_tensor(
                                acc_sb[stile][:, :], acc_sb[stile][:, :], ps_po[:, :],
                                op=ALU.add,
                            )
                        if h == HG - 1:
                            nc.sync.dma_start(
                                out=out[stile * 128 : (stile + 1) * 128, :],
                                in_=acc_sb[stile][:, :],
                            )
            psO.release()
            ps2.release()
            cblkpool.release()
            accpool.release()
            ctxpool.release()
            ppool.release()
            qkvpool.release()

    nc.compile()
    return nc


def _get_nc():
    if "nc" not in _CACHE:
        _CACHE["nc"] = _build()
    return _CACHE["nc"]


def _shard(hidden_states, position_ids, Wq, Wkv, Wo):
    """Pure layout work: slice + transpose per core. No arithmetic."""
    x = np.asarray(hidden_states, np.float32)
    pos = np.asarray(position_ids, np.int32)
    Wq = np.asarray(Wq, np.float32)
    Wkv = np.asarray(Wkv, np.float32)
    Wo = np.asarray(Wo, np.float32)

    in_maps = []
    for c in range(N_CORES):
        b, g = c // TP, c % TP
        krows = g * 2 * HD + 2 * np.arange(HD)
        in_maps.append(
            {
                "xT": np.ascontiguousarray(x[b].T),
                "wqT": np.ascontiguousarray(Wq[g * OC : (g + 1) * OC].T),
                "wkT": np.ascontiguousarray(Wkv[krows].T),
                "wvT": np.ascontiguousarray(Wkv[krows + 1].T),
                "woT": np.ascontiguousarray(Wo[g * OC : (g + 1) * OC].T),
                "pos": np.ascontiguousarray(pos[b][None, :]),
            }
        )
    return in_maps


def run(hidden_states, position_ids, Wq, Wkv, Wo, trace=False):
    nc = _get_nc()
    in_maps = _shard(hidden_states, position_ids, Wq, Wkv, Wo)
    res = bass_utils.run_bass_kernel_spmd(
        nc, in_maps, core_ids=list(range(N_CORES)), trace=trace
    )
    out = np.empty((B, S, HID), np.float32)
    for c in range(N_CORES):
        b, g = c // TP, c % TP
        out[b][:, g * OC : (g + 1) * OC] = res.results[c]["out_slice"]
    return out, res


def kernel(hidden_states, position_ids, Wq, Wkv, Wo):
    out, _ = run(hidden_states, position_ids, Wq, Wkv, Wo, trace=False)
    return out


# revision 10
# speedup vs baseline: 1.1048x; 1.1048x over previous
"""Trainium2 Bass kernel for nn_MultiHeadAttention_3539053052118.

GQA attention (B=2, S=2048, HID=2048, 16 q-heads, 4 kv-heads, RoPE, causal)
distributed over 8 NeuronCores: 2-way data-parallel over batch x 4-way
tensor-parallel over kv-head groups. Each core computes q/kv projections for
its 4 q-heads + 1 kv-head (fp32r matmuls), RoPE, causal flash attention; each
head's context is AllGather-ed (bf16) within the 4-core batch group as soon
as it is ready, and the o_proj accumulates per-wave into SBUF so the
collectives overlap attention. Each core produces a distinct 512-column slice
of the output. The host only shards/aliases inputs and concatenates slices.
"""

import math
import sys
import types

sys.path.insert(0, "/opt/trn_rl_repo")

import antenv  # noqa: F401

if "antenv.axon_hooks" not in sys.modules:
    _hooks = types.ModuleType("antenv.axon_hooks")
    _hook_box = {"hook": None}
    _hooks.set_axon_ntff_profile_hook = lambda h: _hook_box.__setitem__("hook", h)
    _hooks.get_axon_ntff_profile_hook = lambda: _hook_box["hook"]
    sys.modules["antenv.axon_hooks"] = _hooks
    try:
        from trn_agent_boot.trn_boot import _ntff_profile_via_ctypes

        _hooks.set_axon_ntff_profile_hook(
            _ntff_profile_via_ctypes("/opt/axon/libaxon_pjrt.so")
        )
    except Exception:
        pass

import numpy as np
import concourse.bass as bass
import concourse.mybir as mybir
import concourse.tile as tile
from concourse import bacc
from concourse import bass_utils
from concourse.masks import make_identity

F32 = mybir.dt.float32
F32R = mybir.dt.float32r
BF16 = mybir.dt.bfloat16
I32 = mybir.dt.int32
AF = mybir.ActivationFunctionType
ALU = mybir.AluOpType

B, S, HID = 2, 2048, 2048
NH, NKV = 16, 4
HD = 128
ROPE_BASE = 10000.0
PI = math.pi

N_CORES = 8
TP = 4
HG = NH // TP  # 4 q heads per core
GROUPS = [[0, 1, 2, 3], [4, 5, 6, 7]]

NKC = HID // 128  # 16 contraction tiles
NQC = S // 512  # 4 q/n chunks
NST = S // 128  # 16 s tiles
OC = 512  # output columns per core

_CACHE = {}


def _build():
    nc = bacc.Bacc("TRN2", target_bir_lowering=False, debug=False, num_devices=N_CORES)

    xT = nc.dram_tensor("xT", [HID, S], F32R, kind="ExternalInput").ap()
    wqT = nc.dram_tensor("wqT", [HID, HG * HD], F32R, kind="ExternalInput").ap()
    wkT = nc.dram_tensor("wkT", [HID, HD], F32R, kind="ExternalInput").ap()
    wvT = nc.dram_tensor("wvT", [HID, HD], F32R, kind="ExternalInput").ap()
    woT = nc.dram_tensor("woT", [HID, OC], F32, kind="ExternalInput").ap()
    pos = nc.dram_tensor("pos", [1, S], I32, kind="ExternalInput").ap()
    out = nc.dram_tensor("out_slice", [S, OC], F32, kind="ExternalOutput").ap()

    # per-head collective bounce buffers (separate tensors so AG(h) only
    # depends on head h's writes)
    cc_in = [
        [nc.dram_tensor(f"cc_in{h}_{hf}", [HD, S // 2], BF16).ap() for hf in range(2)]
        for h in range(HG)
    ]
    cc_out = [
        [
            nc.dram_tensor(f"cc_out{h}_{hf}", [TP * HD, S // 2], BF16).ap()
            for hf in range(2)
        ]
        for h in range(HG)
    ]

    # ---- inline constants ----
    half = HD // 2
    invf = 1.0 / (ROPE_BASE ** (np.arange(half) / half))
    invf_t = np.tile(invf, 2)[:, None].astype(np.float32) / (2 * PI)  # turns
    invf_c = nc.inline_tensor(invf_t, "invf").ap()
    R = np.zeros((HD, HD), np.float32)
    for p in range(half):
        R[p, p + half] = -1.0
    for p in range(half, HD):
        R[p, p - half] = 1.0
    permRT_c = nc.inline_tensor(np.ascontiguousarray(R.T), "permRT").ap()
    ones_row_c = nc.inline_tensor(np.ones((1, 128), np.float32), "ones_row").ap()
    import ml_dtypes

    ones_col_c = nc.inline_tensor(
        np.ones((128, 1), ml_dtypes.bfloat16), "ones_col"
    ).ap()
    # causal bias: B[p, j] = -30000 where key p > query j (within diag subtile)
    btri = np.where(
        np.arange(128)[:, None] > np.arange(128)[None, :], -30000.0, 0.0
    ).astype(np.float32)
    btriT_c = nc.inline_tensor(np.ascontiguousarray(btri.T), "btriT").ap()
    iext_np = np.zeros((128, 512), np.float32)
    iext_np[:, :128] = np.eye(128, dtype=np.float32)
    iext_c = nc.inline_tensor(iext_np, "iext").ap()

    with tile.TileContext(nc) as tc:
        with (
            tc.tile_pool(name="const", bufs=1) as cpool,
            tc.tile_pool(name="w", bufs=1) as wpool,
            tc.tile_pool(name="attn", bufs=2) as apool,
        ):
            qkvpool = tc.alloc_tile_pool(name="qkv", bufs=1)
            # ---- constants ----
            invf_sb = cpool.tile([HD, 1], F32)
            nc.sync.dma_start(out=invf_sb[:, :], in_=invf_c[:, :])
            permRT_sb = cpool.tile([HD, HD], F32R)
            nc.sync.dma_start(out=permRT_sb[:, :], in_=permRT_c.bitcast(F32R)[:, :])
            ones_row_sb = cpool.tile([1, 128], F32R)
            nc.sync.dma_start(out=ones_row_sb[:, :], in_=ones_row_c.bitcast(F32R)[:, :])
            ones_row_f32 = cpool.tile([1, 128], F32)
            nc.sync.dma_start(out=ones_row_f32[:, :], in_=ones_row_c[:, :])
            ones_col_sb = cpool.tile([128, 1], BF16)
            nc.sync.dma_start(out=ones_col_sb[:, :], in_=ones_col_c[:, :])
            btriT_sb = cpool.tile([128, 128], F32R)
            nc.sync.dma_start(out=btriT_sb[:, :], in_=btriT_c.bitcast(F32R)[:, :])
            iext_sb = cpool.tile([128, 512], F32R)
            nc.sync.dma_start(out=iext_sb[:, :], in_=iext_c.bitcast(F32R)[:, :])
            ident_sb = cpool.tile([128, 128], F32)
            make_identity(nc, ident_sb[:, :])

            # ---- phase 0: rope tables (chunked temps) ----
            psT = tc.alloc_tile_pool(name="psT", bufs=2, space="PSUM")
            tabtmp = tc.alloc_tile_pool(name="tabtmp", bufs=2)
            sinT = cpool.tile([128, S], F32, tag="tab_sin", name="tab_sin")
            cosT = cpool.tile([128, S], F32, tag="tab_cos", name="tab_cos")
            for q in range(NQC):
                ns = slice(q * 512, (q + 1) * 512)
                pos_i = tabtmp.tile([1, 512], I32, tag="pos_i", name=f"pos_i{q}")
                nc.sync.dma_start(out=pos_i[:, :], in_=pos[:, ns])
                pos_f = tabtmp.tile([1, 512], F32R, tag="pos_f", name=f"pos_f{q}")
                nc.vector.tensor_copy(pos_f[:, :], pos_i[:, :])
                ps_pos = psT.tile([128, 512], F32, tag="pos", name=f"ps_pos{q}")
                nc.tensor.matmul(
                    ps_pos[:, :], ones_row_sb[:, :], pos_f[:, :], start=True, stop=True
                )
                for add_quarter, tab in ((False, sinT), (True, cosT)):
                    nm = f"{q}_{int(add_quarter)}"
                    t_t = tabtmp.tile([128, 512], F32, tag="t_t", name=f"t_{nm}")
                    if add_quarter:
                        nc.vector.tensor_scalar(
                            t_t[:, :], ps_pos[:, :], invf_sb[:, :], 0.25,
                            op0=ALU.mult, op1=ALU.add,
                        )
                    else:
                        nc.vector.tensor_scalar_mul(
                            t_t[:, :], ps_pos[:, :], invf_sb[:, :]
                        )
                    t_i = tabtmp.tile([128, 512], I32, tag="t_i", name=f"ti_{nm}")
                    nc.vector.tensor_copy(t_i[:, :], t_t[:, :])
                    t_f = tabtmp.tile([128, 512], F32, tag="t_f", name=f"tf_{nm}")
                    nc.vector.tensor_copy(t_f[:, :], t_i[:, :])
                    nc.vector.tensor_sub(t_t[:, :], t_t[:, :], t_f[:, :])
                    nc.scalar.activation(tab[:, ns], t_t[:, :], AF.Sin, scale=2 * PI)
            tabtmp.release()
            psT.release()

            # ---- weights: wk/wv persistent; wo loaded + cast to bf16 ----
            wk_sb = [
                wpool.tile([128, HD], F32R, tag=f"wk{i}", name=f"wk{i}")
                for i in range(NKC)
            ]
            wv_sb = [
                wpool.tile([128, HD], F32R, tag=f"wv{i}", name=f"wv{i}")
                for i in range(NKC)
            ]
            wo_sb = [
                wpool.tile([128, OC], BF16, tag=f"wo{i}", name=f"wo{i}")
                for i in range(NKC)
            ]
            # persistent qkv storage
            q_sb = [
                qkvpool.tile([128, S], F32R, tag=f"q{h}", name=f"q{h}")
                for h in range(HG)
            ]
            k_sb = qkvpool.tile([128, S], F32R, tag="k", name="k_sb")
            vT_sb = qkvpool.tile([128, S], F32, tag="vT", name="vT_sb")
            v_sb = [
                qkvpool.tile([128, HD], BF16, tag=f"v{i}", name=f"v{i}")
                for i in range(NST)
            ]

            wqpool = tc.alloc_tile_pool(name="wq", bufs=1)
            wq_sb = [
                wqpool.tile([128, HG * HD], F32R, tag=f"wq{i}", name=f"wq{i}")
                for i in range(NKC)
            ]
            xspool = tc.alloc_tile_pool(name="xs", bufs=1)
            psP = tc.alloc_tile_pool(name="psP", bufs=1, space="PSUM")
            psR = tc.alloc_tile_pool(name="psR", bufs=1, space="PSUM")

            # prefetch chunk-0 x before the bulk weight loads
            x_pending = {}
            for kt in range(NKC):
                t = xspool.tile([128, 512], F32R, tag=f"x{kt}", name=f"x_0_{kt}")
                nc.sync.dma_start(out=t[:, :], in_=xT[kt * 128 : (kt + 1) * 128, 0:512])
                x_pending[kt] = t
            for kt in range(NKC):
                sl = slice(kt * 128, (kt + 1) * 128)
                nc.sync.dma_start(out=wq_sb[kt][:, :], in_=wqT[sl, :])
                nc.sync.dma_start(out=wk_sb[kt][:, :], in_=wkT[sl, :])
                nc.sync.dma_start(out=wv_sb[kt][:, :], in_=wvT[sl, :])
            for kt in range(NKC):
                wtmp = apool.tile([128, OC], F32, tag="wotmp", name=f"wotmp{kt}")
                nc.sync.dma_start(out=wtmp[:, :], in_=woT[kt * 128 : (kt + 1) * 128, :])
                nc.vector.tensor_copy(wo_sb[kt][:, :], wtmp[:, :])

            # ---- phase 1: projections + rope + v transpose ----
            for q in range(NQC):
                ns = slice(q * 512, (q + 1) * 512)
                if q == 0:
                    x_sb = [x_pending[kt] for kt in range(NKC)]
                else:
                    x_sb = []
                    for kt in range(NKC):
                        t = xspool.tile(
                            [128, 512], F32R, tag=f"x{kt}", name=f"x_{q}_{kt}"
                        )
                        nc.sync.dma_start(
                            out=t[:, :], in_=xT[kt * 128 : (kt + 1) * 128, ns]
                        )
                        x_sb.append(t)
                ps_proj = [
                    psP.tile([128, 512], F32, tag=f"proj{i}", name=f"proj{i}_{q}")
                    for i in range(HG + 2)
                ]
                for kt in range(NKC):
                    st, sp = kt == 0, kt == NKC - 1
                    for h in range(HG):
                        nc.tensor.matmul(
                            ps_proj[h][:, :],
                            wq_sb[kt][:, h * HD : (h + 1) * HD],
                            x_sb[kt][:, :],
                            start=st,
                            stop=sp,
                        )
                    nc.tensor.matmul(
                        ps_proj[HG][:, :], wk_sb[kt][:, :], x_sb[kt][:, :],
                        start=st, stop=sp,
                    )
                    nc.tensor.matmul(
                        ps_proj[HG + 1][:, :], wv_sb[kt][:, :], x_sb[kt][:, :],
                        start=st, stop=sp,
                    )

                for h in range(HG):
                    nc.vector.tensor_copy(q_sb[h][:, ns], ps_proj[h][:, :])
                nc.vector.tensor_copy(k_sb[:, ns], ps_proj[HG][:, :])
                nc.vector.tensor_copy(vT_sb[:, ns], ps_proj[HG + 1][:, :])

                for idx in range(HG + 1):
                    tgt = q_sb[idx][:, ns] if idx < HG else k_sb[:, ns]
                    ps_rot = psR.tile([128, 512], F32, tag="rot", name=f"rot{q}_{idx}")
                    nc.tensor.matmul(
                        ps_rot[:, :], permRT_sb[:, :], tgt, start=True, stop=True
                    )
                    tmp = apool.tile([128, 512], F32, tag="ropetmp", name=f"rt{q}_{idx}")
                    nc.vector.tensor_tensor(tmp[:, :], tgt, cosT[:, ns], op=ALU.mult)
                    nc.vector.tensor_tensor(tgt, ps_rot[:, :], sinT[:, ns], op=ALU.mult)
                    nc.vector.tensor_tensor(tgt, tgt, tmp[:, :], op=ALU.add)

                for j in range(4):
                    stile = q * 4 + j
                    ps_v = psR.tile([128, 128], F32, tag="vt", name=f"vt{stile}")
                    nc.tensor.transpose(
                        ps_v[:, :],
                        vT_sb[:, stile * 128 : (stile + 1) * 128],
                        ident_sb[:, :],
                    )
                    nc.vector.tensor_copy(v_sb[stile][:, :], ps_v[:, :])
            psR.release()
            psP.release()
            xspool.release()
            wqpool.release()

            # ---- phase 2: attention; AG(h) issued per head; o_proj waves ----
            ppool = tc.alloc_tile_pool(name="probs", bufs=3)
            ctxpool = tc.alloc_tile_pool(name="ctx", bufs=2)
            accpool = tc.alloc_tile_pool(name="acc", bufs=1)
            cblkpool = tc.alloc_tile_pool(name="cblk", bufs=1)
            ps2 = tc.alloc_tile_pool(name="ps2", bufs=1, space="PSUM")
            psO = tc.alloc_tile_pool(name="psO", bufs=2, space="PSUM")

            scale = float(HD**-0.5)
            anchors = {}
            for h in range(HG):
                for q in range(NQC):
                    ns = slice(q * 512, (q + 1) * 512)
                    nkt = 4 * q + 4
                    ps_sums = ps2.tile(
                        [1, 512], F32, tag="sums", name=f"sums{h}_{q}", bufs=2
                    )
                    ps_ctx = ps2.tile(
                        [128, 512], F32, tag="ctx", name=f"ctx{h}_{q}", bufs=2
                    )
                    for kt in range(nkt):
                        o = kt - 4 * q
                        c0 = max(0, o) * 128  # first valid column in the chunk
                        cs = slice(c0, 512)
                        ps_s = ps2.tile(
                            [128, 512], F32, tag="scores", name=f"s{h}_{q}_{kt}", bufs=2
                        )
                        if o >= 0:
                            # causal bias written into the bank, scores accumulate
                            nc.tensor.matmul(
                                ps_s[:, cs],
                                btriT_sb[:, :],
                                iext_sb[:, 0 : 512 - c0],
                                start=True,
                                stop=False,
                            )
                        nc.tensor.matmul(
                            ps_s[:, cs],
                            k_sb[:, kt * 128 : (kt + 1) * 128],
                            q_sb[h][:, q * 512 + c0 : (q + 1) * 512],
                            start=o < 0,
                            stop=True,
                        )
                        pT = ppool.tile(
                            [128, 512], BF16, tag="probs", name=f"p{h}_{q}_{kt}"
                        )
                        nc.scalar.activation(pT[:, cs], ps_s[:, cs], AF.Exp, scale=scale)
                        st, sp = kt == 0, kt == nkt - 1
                        nc.tensor.matmul(
                            ps_sums[:, cs], ones_col_sb[:, :], pT[:, cs],
                            start=st, stop=sp,
                        )
                        nc.tensor.matmul(
                            ps_ctx[:, cs], v_sb[kt][:, :], pT[:, cs],
                            start=st, stop=sp,
                        )
                    # free the psum banks fast; normalize off the PE path
                    ctx_raw = apool.tile(
                        [128, 512], F32, tag="ctx_raw", name=f"cr{h}_{q}"
                    )
                    nc.vector.tensor_copy(ctx_raw[:, :], ps_ctx[:, :])
                    recip = apool.tile([1, 512], F32, tag="recip", name=f"rc{h}_{q}")
                    nc.vector.reciprocal_approx_fast(recip[:, :], ps_sums[:, :])
                    ps_rb = ps2.tile(
                        [128, 512], F32, tag="scores", name=f"rb{h}_{q}", bufs=2
                    )
                    nc.tensor.matmul(
                        ps_rb[:, :], ones_row_f32[:, :], recip[:, :], start=True, stop=True
                    )
                    csb = ctxpool.tile([128, 512], BF16, tag="ctxsb", name=f"cs{h}_{q}")
                    nc.vector.tensor_tensor(
                        csb[:, :], ctx_raw[:, :], ps_rb[:, :], op=ALU.mult
                    )
                    hf = q // 2
                    csb_dma = nc.sync.dma_start(
                        out=cc_in[h][hf][:, (q % 2) * 512 : (q % 2 + 1) * 512],
                        in_=csb[:, :],
                    )
                    anchors[(h, q)] = csb_dma.ins
                    if q % 2 == 1:
                        # half of head h's context is done on all ranks
                        nc.gpsimd.collective_compute(
                            "AllGather",
                            mybir.AluOpType.bypass,
                            replica_groups=GROUPS,
                            ins=[cc_in[h][hf][:, :]],
                            outs=[cc_out[h][hf][:, :]],
                        )

            # o_proj waves: wave h brings rows for global kt = 4r + h
            acc_sb = [
                accpool.tile([128, OC], F32, tag=f"acc{i}", name=f"acc{i}")
                for i in range(NST)
            ]
            for h in range(HG):
                for hf in range(2):
                    anchor = anchors.get((h + 1, 2 * hf + 1))
                    cblk = []
                    for r in range(TP):
                        t = cblkpool.tile(
                            [128, S // 2], BF16, tag=f"cblk{r}",
                            name=f"cb{h}_{hf}_{r}", bufs=2,
                        )
                        d = nc.sync.dma_start(
                            out=t[:, :], in_=cc_out[h][hf][r * 128 : (r + 1) * 128, :]
                        )
                        if anchor is not None:
                            tile.add_dep_helper(
                                d.ins, anchor, False, "delay o_proj wave"
                            )
                        cblk.append(t)
                    for j in range(NST // 2):
                        stile = hf * (NST // 2) + j
                        ps_po = psO.tile(
                            [128, OC], F32, tag="po", name=f"po{h}_{hf}_{j}"
                        )
                        for r in range(TP):
                            nc.tensor.matmul(
                                ps_po[:, :],
                                cblk[r][:, j * 128 : (j + 1) * 128],
                                wo_sb[4 * r + h][:, :],
                                start=r == 0,
                                stop=r == TP - 1,
                            )
                        if h == 0:
                            nc.vector.tensor_copy(acc_sb[stile][:, :], ps_po[:, :])
                        else:
                            nc.vector.tensor_tensor(
                                acc_sb[stile][:, :], acc_sb[stile][:, :], ps_po[:, :],
                                op=ALU.add,
                            )
                        if h == HG - 1:
                            nc.sync.dma_start(
                                out=out[stile * 128 : (stile + 1) * 128, :],
                                in_=acc_sb[stile][:, :],
                            )
            psO.release()
            ps2.release()
            cblkpool.release()
            accpool.release()
            ctxpool.release()
            ppool.release()
            qkvpool.release()

    nc.compile()
    return nc


def _get_nc():
    if "nc" not in _CACHE:
        _CACHE["nc"] = _build()
    return _CACHE["nc"]


def _shard(hidden_states, position_ids, Wq, Wkv, Wo):
    """Pure layout work: slice + transpose per core. No arithmetic."""
    x = np.asarray(hidden_states, np.float32)
    pos = np.asarray(position_ids, np.int32)
    Wq = np.asarray(Wq, np.float32)
    Wkv = np.asarray(Wkv, np.float32)
    Wo = np.asarray(Wo, np.float32)

    in_maps = []
    for c in range(N_CORES):
        b, g = c // TP, c % TP
        krows = g * 2 * HD + 2 * np.arange(HD)
        in_maps.append(
            {
                "xT": np.ascontiguousarray(x[b].T),
                "wqT": np.ascontiguousarray(Wq[g * OC : (g + 1) * OC].T),
                "wkT": np.ascontiguousarray(Wkv[krows].T),
                "wvT": np.ascontiguousarray(Wkv[krows + 1].T),
                "woT": np.ascontiguousarray(Wo[g * OC : (g + 1) * OC].T),
                "pos": np.ascontiguousarray(pos[b][None, :]),
            }
        )
    return in_maps


def run(hidden_states, position_ids, Wq, Wkv, Wo, trace=False):
    nc = _get_nc()
    in_maps = _shard(hidden_states, position_ids, Wq, Wkv, Wo)
    res = bass_utils.run_bass_kernel_spmd(
        nc, in_maps, core_ids=list(range(N_CORES)), trace=trace
    )
    out = np.empty((B, S, HID), np.float32)
    for c in range(N_CORES):
        b, g = c // TP, c % TP
        out[b][:, g * OC : (g + 1) * OC] = res.results[c]["out_slice"]
    return out, res


def kernel(hidden_states, position_ids, Wq, Wkv, Wo):
    out, _ = run(hidden_states, position_ids, Wq, Wkv, Wo, trace=False)
    return out


# revision 11
# speedup vs baseline: 1.1509x; 1.0417x over previous
"""Trainium2 Bass kernel for nn_MultiHeadAttention_3539053052118.

GQA attention (B=2, S=2048, HID=2048, 16 q-heads, 4 kv-heads, RoPE, causal)
distributed over 8 NeuronCores: 2-way data-parallel over batch x 4-way
tensor-parallel over kv-head groups. Each core computes q/kv projections for
its 4 q-heads + 1 kv-head (fp32r matmuls), RoPE, causal flash attention; each
head's context is AllGather-ed (bf16) within the 4-core batch group as soon
as it is ready, and the o_proj accumulates per-wave into SBUF so the
collectives overlap attention. Each core produces a distinct 512-column slice
of the output. The host only shards/aliases inputs and concatenates slices.
"""

import math
import sys
import types

sys.path.insert(0, "/opt/trn_rl_repo")

import antenv  # noqa: F401

if "antenv.axon_hooks" not in sys.modules:
    _hooks = types.ModuleType("antenv.axon_hooks")
    _hook_box = {"hook": None}
    _hooks.set_axon_ntff_profile_hook = lambda h: _hook_box.__setitem__("hook", h)
    _hooks.get_axon_ntff_profile_hook = lambda: _hook_box["hook"]
    sys.modules["antenv.axon_hooks"] = _hooks
    try:
        from trn_agent_boot.trn_boot import _ntff_profile_via_ctypes

        _hooks.set_axon_ntff_profile_hook(
            _ntff_profile_via_ctypes("/opt/axon/libaxon_pjrt.so")
        )
    except Exception:
        pass

import numpy as np
import concourse.bass as bass
import concourse.mybir as mybir
import concourse.tile as tile
from concourse import bacc
from concourse import bass_utils
from concourse.masks import make_identity

F32 = mybir.dt.float32
F32R = mybir.dt.float32r
BF16 = mybir.dt.bfloat16
I32 = mybir.dt.int32
AF = mybir.ActivationFunctionType
ALU = mybir.AluOpType

B, S, HID = 2, 2048, 2048
NH, NKV = 16, 4
HD = 128
ROPE_BASE = 10000.0
PI = math.pi

N_CORES = 8
TP = 4
HG = NH // TP  # 4 q heads per core
GROUPS = [[0, 1, 2, 3], [4, 5, 6, 7]]

NKC = HID // 128  # 16 contraction tiles
NQC = S // 512  # 4 q/n chunks
NST = S // 128  # 16 s tiles
OC = 512  # output columns per core

_CACHE = {}


def _build():
    nc = bacc.Bacc("TRN2", target_bir_lowering=False, debug=False, num_devices=N_CORES)

    xT = nc.dram_tensor("xT", [HID, S], F32R, kind="ExternalInput").ap()
    wqT = nc.dram_tensor("wqT", [HID, HG * HD], F32R, kind="ExternalInput").ap()
    wkT = nc.dram_tensor("wkT", [HID, HD], F32R, kind="ExternalInput").ap()
    wvT = nc.dram_tensor("wvT", [HID, HD], F32R, kind="ExternalInput").ap()
    woT = nc.dram_tensor("woT", [HID, OC], F32, kind="ExternalInput").ap()
    pos = nc.dram_tensor("pos", [1, S], I32, kind="ExternalInput").ap()
    out = nc.dram_tensor("out_slice", [S, OC], F32, kind="ExternalOutput").ap()

    # per-head collective bounce buffers (separate tensors so AG(h) only
    # depends on head h's writes)
    cc_in = [
        [nc.dram_tensor(f"cc_in{h}_{hf}", [HD, S // 2], BF16).ap() for hf in range(2)]
        for h in range(HG)
    ]
    cc_out = [
        [
            nc.dram_tensor(f"cc_out{h}_{hf}", [TP * HD, S // 2], BF16).ap()
            for hf in range(2)
        ]
        for h in range(HG)
    ]

    # ---- inline constants ----
    half = HD // 2
    invf = 1.0 / (ROPE_BASE ** (np.arange(half) / half))
    invf_t = np.tile(invf, 2)[:, None].astype(np.float32) / (2 * PI)  # turns
    invf_c = nc.inline_tensor(invf_t, "invf").ap()
    R = np.zeros((HD, HD), np.float32)
    for p in range(half):
        R[p, p + half] = -1.0
    for p in range(half, HD):
        R[p, p - half] = 1.0
    permRT_c = nc.inline_tensor(np.ascontiguousarray(R.T), "permRT").ap()
    ones_row_c = nc.inline_tensor(np.ones((1, 128), np.float32), "ones_row").ap()
    import ml_dtypes

    ones_col_c = nc.inline_tensor(
        np.ones((128, 1), ml_dtypes.bfloat16), "ones_col"
    ).ap()
    # causal bias: B[p, j] = -30000 where key p > query j (within diag subtile)
    btri = np.where(
        np.arange(128)[:, None] > np.arange(128)[None, :], -30000.0, 0.0
    ).astype(np.float32)
    btriT_c = nc.inline_tensor(np.ascontiguousarray(btri.T), "btriT").ap()
    iext_np = np.zeros((128, 512), np.float32)
    iext_np[:, :128] = np.eye(128, dtype=np.float32)
    iext_c = nc.inline_tensor(iext_np, "iext").ap()

    with tile.TileContext(nc) as tc:
        with (
            tc.tile_pool(name="const", bufs=1) as cpool,
            tc.tile_pool(name="w", bufs=1) as wpool,
            tc.tile_pool(name="attn", bufs=2) as apool,
        ):
            qkvpool = tc.alloc_tile_pool(name="qkv", bufs=1)
            # ---- constants ----
            invf_sb = cpool.tile([HD, 1], F32)
            nc.sync.dma_start(out=invf_sb[:, :], in_=invf_c[:, :])
            permRT_sb = cpool.tile([HD, HD], F32R)
            nc.sync.dma_start(out=permRT_sb[:, :], in_=permRT_c.bitcast(F32R)[:, :])
            ones_row_sb = cpool.tile([1, 128], F32R)
            nc.sync.dma_start(out=ones_row_sb[:, :], in_=ones_row_c.bitcast(F32R)[:, :])
            ones_row_f32 = cpool.tile([1, 128], F32)
            nc.sync.dma_start(out=ones_row_f32[:, :], in_=ones_row_c[:, :])
            ones_col_sb = cpool.tile([128, 1], BF16)
            nc.sync.dma_start(out=ones_col_sb[:, :], in_=ones_col_c[:, :])
            btriT_sb = cpool.tile([128, 128], F32R)
            nc.sync.dma_start(out=btriT_sb[:, :], in_=btriT_c.bitcast(F32R)[:, :])
            iext_sb = cpool.tile([128, 512], F32R)
            nc.sync.dma_start(out=iext_sb[:, :], in_=iext_c.bitcast(F32R)[:, :])
            ident_sb = cpool.tile([128, 128], F32)
            make_identity(nc, ident_sb[:, :])

            # ---- phase 0: rope tables (chunked temps) ----
            psT = tc.alloc_tile_pool(name="psT", bufs=2, space="PSUM")
            tabtmp = tc.alloc_tile_pool(name="tabtmp", bufs=2)
            sinT = cpool.tile([128, S], F32, tag="tab_sin", name="tab_sin")
            cosT = cpool.tile([128, S], F32, tag="tab_cos", name="tab_cos")
            for q in range(NQC):
                ns = slice(q * 512, (q + 1) * 512)
                pos_i = tabtmp.tile([1, 512], I32, tag="pos_i", name=f"pos_i{q}")
                nc.sync.dma_start(out=pos_i[:, :], in_=pos[:, ns])
                pos_f = tabtmp.tile([1, 512], F32R, tag="pos_f", name=f"pos_f{q}")
                nc.vector.tensor_copy(pos_f[:, :], pos_i[:, :])
                ps_pos = psT.tile([128, 512], F32, tag="pos", name=f"ps_pos{q}")
                nc.tensor.matmul(
                    ps_pos[:, :], ones_row_sb[:, :], pos_f[:, :], start=True, stop=True
                )
                for add_quarter, tab in ((False, sinT), (True, cosT)):
                    nm = f"{q}_{int(add_quarter)}"
                    t_t = tabtmp.tile([128, 512], F32, tag="t_t", name=f"t_{nm}")
                    if add_quarter:
                        nc.vector.tensor_scalar(
                            t_t[:, :], ps_pos[:, :], invf_sb[:, :], 0.25,
                            op0=ALU.mult, op1=ALU.add,
                        )
                    else:
                        nc.vector.tensor_scalar_mul(
                            t_t[:, :], ps_pos[:, :], invf_sb[:, :]
                        )
                    t_i = tabtmp.tile([128, 512], I32, tag="t_i", name=f"ti_{nm}")
                    nc.vector.tensor_copy(t_i[:, :], t_t[:, :])
                    t_f = tabtmp.tile([128, 512], F32, tag="t_f", name=f"tf_{nm}")
                    nc.vector.tensor_copy(t_f[:, :], t_i[:, :])
                    nc.vector.tensor_sub(t_t[:, :], t_t[:, :], t_f[:, :])
                    nc.scalar.activation(tab[:, ns], t_t[:, :], AF.Sin, scale=2 * PI)
            tabtmp.release()
            psT.release()

            # ---- weights: wk/wv persistent; wo loaded + cast to bf16 ----
            wk_sb = [
                wpool.tile([128, HD], F32R, tag=f"wk{i}", name=f"wk{i}")
                for i in range(NKC)
            ]
            wv_sb = [
                wpool.tile([128, HD], F32R, tag=f"wv{i}", name=f"wv{i}")
                for i in range(NKC)
            ]
            wo_sb = [
                wpool.tile([128, OC], BF16, tag=f"wo{i}", name=f"wo{i}")
                for i in range(NKC)
            ]
            # persistent qkv storage
            q_sb = [
                qkvpool.tile([128, S], F32R, tag=f"q{h}", name=f"q{h}")
                for h in range(HG)
            ]
            k_sb = qkvpool.tile([128, S], F32R, tag="k", name="k_sb")
            vT_sb = qkvpool.tile([128, S], F32, tag="vT", name="vT_sb")
            v_sb = [
                qkvpool.tile([128, HD], BF16, tag=f"v{i}", name=f"v{i}")
                for i in range(NST)
            ]

            wqpool = tc.alloc_tile_pool(name="wq", bufs=1)
            wq_sb = [
                wqpool.tile([128, HG * HD], F32R, tag=f"wq{i}", name=f"wq{i}")
                for i in range(NKC)
            ]
            xspool = tc.alloc_tile_pool(name="xs", bufs=1)
            psP = tc.alloc_tile_pool(name="psP", bufs=1, space="PSUM")
            psR = tc.alloc_tile_pool(name="psR", bufs=1, space="PSUM")

            # prefetch chunk-0 x before the bulk weight loads
            x_pending = {}
            for kt in range(NKC):
                t = xspool.tile([128, 512], F32R, tag=f"x{kt}", name=f"x_0_{kt}")
                nc.sync.dma_start(out=t[:, :], in_=xT[kt * 128 : (kt + 1) * 128, 0:512])
                x_pending[kt] = t
            for kt in range(NKC):
                sl = slice(kt * 128, (kt + 1) * 128)
                nc.sync.dma_start(out=wq_sb[kt][:, :], in_=wqT[sl, :])
                nc.sync.dma_start(out=wk_sb[kt][:, :], in_=wkT[sl, :])
                nc.sync.dma_start(out=wv_sb[kt][:, :], in_=wvT[sl, :])
            # ---- phase 1: projections + rope + v transpose ----
            for q in range(NQC):
                ns = slice(q * 512, (q + 1) * 512)
                if q == 0:
                    x_sb = [x_pending[kt] for kt in range(NKC)]
                else:
                    x_sb = []
                    for kt in range(NKC):
                        t = xspool.tile(
                            [128, 512], F32R, tag=f"x{kt}", name=f"x_{q}_{kt}"
                        )
                        nc.sync.dma_start(
                            out=t[:, :], in_=xT[kt * 128 : (kt + 1) * 128, ns]
                        )
                        x_sb.append(t)
                ps_proj = [
                    psP.tile([128, 512], F32, tag=f"proj{i}", name=f"proj{i}_{q}")
                    for i in range(HG + 2)
                ]
                for kt in range(NKC):
                    st, sp = kt == 0, kt == NKC - 1
                    for h in range(HG):
                        nc.tensor.matmul(
                            ps_proj[h][:, :],
                            wq_sb[kt][:, h * HD : (h + 1) * HD],
                            x_sb[kt][:, :],
                            start=st,
                            stop=sp,
                        )
                    nc.tensor.matmul(
                        ps_proj[HG][:, :], wk_sb[kt][:, :], x_sb[kt][:, :],
                        start=st, stop=sp,
                    )
                    nc.tensor.matmul(
                        ps_proj[HG + 1][:, :], wv_sb[kt][:, :], x_sb[kt][:, :],
                        start=st, stop=sp,
                    )

                for h in range(HG):
                    nc.vector.tensor_copy(q_sb[h][:, ns], ps_proj[h][:, :])
                nc.vector.tensor_copy(k_sb[:, ns], ps_proj[HG][:, :])
                nc.vector.tensor_copy(vT_sb[:, ns], ps_proj[HG + 1][:, :])

                for idx in range(HG + 1):
                    tgt = q_sb[idx][:, ns] if idx < HG else k_sb[:, ns]
                    ps_rot = psR.tile([128, 512], F32, tag="rot", name=f"rot{q}_{idx}")
                    nc.tensor.matmul(
                        ps_rot[:, :], permRT_sb[:, :], tgt, start=True, stop=True
                    )
                    tmp = apool.tile([128, 512], F32, tag="ropetmp", name=f"rt{q}_{idx}")
                    nc.vector.tensor_tensor(tmp[:, :], tgt, cosT[:, ns], op=ALU.mult)
                    nc.vector.tensor_tensor(tgt, ps_rot[:, :], sinT[:, ns], op=ALU.mult)
                    nc.vector.tensor_tensor(tgt, tgt, tmp[:, :], op=ALU.add)

                for j in range(4):
                    stile = q * 4 + j
                    ps_v = psR.tile([128, 128], F32, tag="vt", name=f"vt{stile}")
                    nc.tensor.transpose(
                        ps_v[:, :],
                        vT_sb[:, stile * 128 : (stile + 1) * 128],
                        ident_sb[:, :],
                    )
                    nc.vector.tensor_copy(v_sb[stile][:, :], ps_v[:, :])
            psR.release()
            psP.release()
            xspool.release()
            wqpool.release()

            # o_proj weights: load + cast now (overlaps attention)
            for kt in range(NKC):
                wtmp = apool.tile([128, OC], F32, tag="wotmp", name=f"wotmp{kt}")
                nc.sync.dma_start(out=wtmp[:, :], in_=woT[kt * 128 : (kt + 1) * 128, :])
                nc.vector.tensor_copy(wo_sb[kt][:, :], wtmp[:, :])

            # ---- phase 2: attention; AG(h) issued per head; o_proj waves ----
            ppool = tc.alloc_tile_pool(name="probs", bufs=3)
            ctxpool = tc.alloc_tile_pool(name="ctx", bufs=2)
            accpool = tc.alloc_tile_pool(name="acc", bufs=1)
            cblkpool = tc.alloc_tile_pool(name="cblk", bufs=1)
            ps2 = tc.alloc_tile_pool(name="ps2", bufs=1, space="PSUM")
            psO = tc.alloc_tile_pool(name="psO", bufs=2, space="PSUM")

            scale = float(HD**-0.5)
            anchors = {}
            pending = None
            for h in range(HG):
                for q in range(NQC):
                    ns = slice(q * 512, (q + 1) * 512)
                    nkt = 4 * q + 4
                    ps_sums = ps2.tile(
                        [1, 512], F32, tag="sums", name=f"sums{h}_{q}", bufs=2
                    )
                    ps_ctx = ps2.tile(
                        [128, 512], F32, tag="ctx", name=f"ctx{h}_{q}", bufs=2
                    )
                    for kt in range(nkt):
                        o = kt - 4 * q
                        c0 = max(0, o) * 128  # first valid column in the chunk
                        cs = slice(c0, 512)
                        ps_s = ps2.tile(
                            [128, 512], F32, tag="scores", name=f"s{h}_{q}_{kt}", bufs=2
                        )
                        if o >= 0:
                            # causal bias written into the bank, scores accumulate
                            nc.tensor.matmul(
                                ps_s[:, cs],
                                btriT_sb[:, :],
                                iext_sb[:, 0 : 512 - c0],
                                start=True,
                                stop=False,
                            )
                        nc.tensor.matmul(
                            ps_s[:, cs],
                            k_sb[:, kt * 128 : (kt + 1) * 128],
                            q_sb[h][:, q * 512 + c0 : (q + 1) * 512],
                            start=o < 0,
                            stop=True,
                        )
                        pT = ppool.tile(
                            [128, 512], BF16, tag="probs", name=f"p{h}_{q}_{kt}"
                        )
                        nc.scalar.activation(pT[:, cs], ps_s[:, cs], AF.Exp, scale=scale)
                        st, sp = kt == 0, kt == nkt - 1
                        nc.tensor.matmul(
                            ps_sums[:, cs], ones_col_sb[:, :], pT[:, cs],
                            start=st, stop=sp,
                        )
                        nc.tensor.matmul(
                            ps_ctx[:, cs], v_sb[kt][:, :], pT[:, cs],
                            start=st, stop=sp,
                        )
                    # free the psum banks fast; broadcast deferred one chunk
                    ctx_raw = apool.tile(
                        [128, 512], F32, tag="ctx_raw", name=f"cr{h}_{q}"
                    )
                    nc.vector.tensor_copy(ctx_raw[:, :], ps_ctx[:, :])
                    recip = apool.tile([1, 512], F32, tag="recip", name=f"rc{h}_{q}")
                    nc.vector.reciprocal_approx_fast(recip[:, :], ps_sums[:, :])

                    def emit_norm(hh, qq, craw, rc):
                        ps_rb = ps2.tile(
                            [128, 512], F32, tag="scores", name=f"rb{hh}_{qq}", bufs=2
                        )
                        nc.tensor.matmul(
                            ps_rb[:, :], ones_row_f32[:, :], rc[:, :],
                            start=True, stop=True,
                        )
                        csb = ctxpool.tile(
                            [128, 512], BF16, tag="ctxsb", name=f"cs{hh}_{qq}"
                        )
                        nc.vector.tensor_tensor(
                            csb[:, :], craw[:, :], ps_rb[:, :], op=ALU.mult
                        )
                        hhf = qq // 2
                        csb_dma = nc.sync.dma_start(
                            out=cc_in[hh][hhf][:, (qq % 2) * 512 : (qq % 2 + 1) * 512],
                            in_=csb[:, :],
                        )
                        anchors[(hh, qq)] = csb_dma.ins
                        if qq % 2 == 1:
                            nc.gpsimd.collective_compute(
                                "AllGather",
                                mybir.AluOpType.bypass,
                                replica_groups=GROUPS,
                                ins=[cc_in[hh][hhf][:, :]],
                                outs=[cc_out[hh][hhf][:, :]],
                            )

                    if pending is not None:
                        emit_norm(*pending)
                    pending = (h, q, ctx_raw, recip)
                    if q == NQC - 1:
                        emit_norm(*pending)
                        pending = None

            # o_proj waves: wave h brings rows for global kt = 4r + h
            acc_sb = [
                accpool.tile([128, OC], F32, tag=f"acc{i}", name=f"acc{i}")
                for i in range(NST)
            ]
            for h in range(HG):
                for hf in range(2):
                    anchor = anchors.get((h + 1, 2 * hf + 1))
                    cblk = []
                    for r in range(TP):
                        t = cblkpool.tile(
                            [128, S // 2], BF16, tag=f"cblk{r}",
                            name=f"cb{h}_{hf}_{r}", bufs=2,
                        )
                        d = nc.sync.dma_start(
                            out=t[:, :], in_=cc_out[h][hf][r * 128 : (r + 1) * 128, :]
                        )
                        if anchor is not None:
                            tile.add_dep_helper(
                                d.ins, anchor, False, "delay o_proj wave"
                            )
                        cblk.append(t)
                    for j in range(NST // 2):
                        stile = hf * (NST // 2) + j
                        ps_po = psO.tile(
                            [128, OC], F32, tag="po", name=f"po{h}_{hf}_{j}"
                        )
                        for r in range(TP):
                            nc.tensor.matmul(
                                ps_po[:, :],
                                cblk[r][:, j * 128 : (j + 1) * 128],
                                wo_sb[4 * r + h][:, :],
                                start=r == 0,
                                stop=r == TP - 1,
                            )
                        if h == 0:
                            nc.vector.tensor_copy(acc_sb[stile][:, :], ps_po[:, :])
                        else:
                            nc.vector.tensor_tensor(
                                acc_sb[stile][:, :], acc_sb[stile][:, :], ps_po[:, :],
                                op=ALU.add,
                            )
                        if h == HG - 1:
                            nc.sync.dma_start(
                                out=out[stile * 128 : (stile + 1) * 128, :],
                                in_=acc_sb[stile][:, :],
                            )
            psO.release()
            ps2.release()
            cblkpool.release()
            accpool.release()
            ctxpool.release()
            ppool.release()
            qkvpool.release()

    nc.compile()
    return nc


def _get_nc():
    if "nc" not in _CACHE:
        _CACHE["nc"] = _build()
    return _CACHE["nc"]


def _shard(hidden_states, position_ids, Wq, Wkv, Wo):
    """Pure layout work: slice + transpose per core. No arithmetic."""
    x = np.asarray(hidden_states, np.float32)
    pos = np.asarray(position_ids, np.int32)
    Wq = np.asarray(Wq, np.float32)
    Wkv = np.asarray(Wkv, np.float32)
    Wo = np.asarray(Wo, np.float32)

    in_maps = []
    for c in range(N_CORES):
        b, g = c // TP, c % TP
        krows = g * 2 * HD + 2 * np.arange(HD)
        in_maps.append(
            {
                "xT": np.ascontiguousarray(x[b].T),
                "wqT": np.ascontiguousarray(Wq[g * OC : (g + 1) * OC].T),
                "wkT": np.ascontiguousarray(Wkv[krows].T),
                "wvT": np.ascontiguousarray(Wkv[krows + 1].T),
                "woT": np.ascontiguousarray(Wo[g * OC : (g + 1) * OC].T),
                "pos": np.ascontiguousarray(pos[b][None, :]),
            }
        )
    return in_maps


def run(hidden_states, position_ids, Wq, Wkv, Wo, trace=False):
    nc = _get_nc()
    in_maps = _shard(hidden_states, position_ids, Wq, Wkv, Wo)
    res = bass_utils.run_bass_kernel_spmd(
        nc, in_maps, core_ids=list(range(N_CORES)), trace=trace
    )
    out = np.empty((B, S, HID), np.float32)
    for c in range(N_CORES):
        b, g = c // TP, c % TP
        out[b][:, g * OC : (g + 1) * OC] = res.results[c]["out_slice"]
    return out, res


def kernel(hidden_states, position_ids, Wq, Wkv, Wo):
    out, _ = run(hidden_states, position_ids, Wq, Wkv, Wo, trace=False)
    return out
